# revision 7
# baseline (speedup 1.0000x reference)
"""Chamfer + edge + normal-cosine combined loss on 8 Trainium2 cores.

Device computes, per core (b = core//2, t-half h = core%2), the negated
distance matrix M[t,p] = 2<gts_t, preds_p> - |gts_t|^2 - |preds_p|^2 = -P[t,p]
via K=5 matmuls, then:
  - m64[t, w]   = max over p in 64-window w of M[t, :]   (row maxima -> mins2/argmin)
  - colmax[q,p] = max over t-tiles of M[t, p]            (-> mins1 after partition reduce)
Host finishes: mins2 = -max_w m64, argmin resolved exactly inside the winning
64-window, mins1 = -colmax reduced across the two t-halves, then the (tiny)
edge/normal-cosine losses in numpy.
"""

from contextlib import ExitStack

import numpy as np

B = 4
N = 8192
NCORES = 8
TH = N // 2          # t rows per core
T_TILES = TH // 128  # 32
P_GROUPS = N // 2048  # 4
WIN = 64
NW = N // WIN        # 128 windows per row
NEG_INF = float(np.finfo(np.float32).min)

_LAST_RESULTS = {}


def _build_nc():
    import concourse.mybir as mybir
    import concourse.tile as tile
    from concourse import bacc, bass_isa

    f32 = mybir.dt.float32
    nc = bacc.Bacc("TRN2", target_bir_lowering=False, debug=False)

    lhsT_d = nc.dram_tensor("lhsT", [5, TH], f32, kind="ExternalInput")
    rhs_d = nc.dram_tensor("rhs", [5, N], f32, kind="ExternalInput")
    m64_d = nc.dram_tensor("m64", [T_TILES, 128, NW], f32, kind="ExternalOutput")
    mins1_d = nc.dram_tensor("mins1neg", [1, N], f32, kind="ExternalOutput")

    with tile.TileContext(nc) as tc, ExitStack() as ctx:
        const_pool = ctx.enter_context(tc.tile_pool(name="const", bufs=1))
        acc_pool = ctx.enter_context(tc.tile_pool(name="acc", bufs=1))
        stage_pool = ctx.enter_context(tc.tile_pool(name="stage", bufs=3))
        psum_pool = ctx.enter_context(tc.tile_pool(name="psum", bufs=2, space="PSUM"))

        lhsT_s = const_pool.tile([5, TH], f32)
        rhs_s = const_pool.tile([5, N], f32)
        nc.sync.dma_start(lhsT_s[:], lhsT_d[:, :])
        nc.sync.dma_start(rhs_s[:], rhs_d[:, :])

        acc = acc_pool.tile([128, N], f32)
        accr = acc_pool.tile([128, N], f32)
        nc.vector.memset(acc[:], NEG_INF)

        for i in range(T_TILES):
            w_i = lhsT_s[:, i * 128 : (i + 1) * 128]
            stage = stage_pool.tile([128, NW], f32)
            for g in range(P_GROUPS):
                ps = psum_pool.tile([128, 2048], f32)
                for c in range(4):
                    nc.tensor.matmul(
                        ps[:, c * 512 : (c + 1) * 512],
                        w_i,
                        rhs_s[:, g * 2048 + c * 512 : g * 2048 + (c + 1) * 512],
                        start=True,
                        stop=True,
                    )
                # per-64-window row maxima of this 2048-wide group
                nc.vector.tensor_reduce(
                    stage[:, g * 32 : (g + 1) * 32],
                    ps[:].rearrange("p (w e) -> p w e", e=WIN),
                    axis=mybir.AxisListType.X,
                    op=mybir.AluOpType.max,
                )
                # column-direction running max for mins1
                nc.vector.tensor_max(
                    acc[:, g * 2048 : (g + 1) * 2048],
                    ps[:],
                    acc[:, g * 2048 : (g + 1) * 2048],
                )
            nc.sync.dma_start(m64_d[i, :, :], stage[:])

        nc.gpsimd.partition_all_reduce(
            accr[:], acc[:], channels=128, reduce_op=bass_isa.ReduceOp.max
        )
        nc.sync.dma_start(mins1_d[:, :], accr[0:1, :])

    nc.compile()
    return nc


def _make_in_maps(preds, gts):
    xsq = np.sum(gts * gts, axis=-1)    # [B, N]
    ysq = np.sum(preds * preds, axis=-1)  # [B, N]
    in_maps = []
    for core in range(NCORES):
        b, h = divmod(core, 2)
        tsl = slice(h * TH, (h + 1) * TH)
        lhsT = np.empty((5, TH), np.float32)
        lhsT[0:3] = (2.0 * gts[b, tsl]).T
        lhsT[3] = xsq[b, tsl]
        lhsT[4] = 1.0
        rhs = np.empty((5, N), np.float32)
        rhs[0:3] = preds[b].T
        rhs[3] = -1.0
        rhs[4] = -ysq[b]
        in_maps.append({"lhsT": lhsT, "rhs": rhs})
    return in_maps


def _postprocess(preds, gts, normals, edges, results):
    xsq = np.sum(gts * gts, axis=-1)
    ysq = np.sum(preds * preds, axis=-1)

    # m64 windows: core (b,h) tile i row q window w -> t = h*TH + i*128 + q
    m64 = np.stack([results[c]["m64"] for c in range(NCORES)])  # [8, 32, 128, NW]
    Mwin = m64.reshape(B, 2, T_TILES, 128, NW).reshape(B, N, NW)
    mins2 = -Mwin.max(axis=2)                  # [B, N]
    wstar = Mwin.argmax(axis=2)                # first occurrence

    m1 = np.stack([results[c]["mins1neg"][0] for c in range(NCORES)])  # [8, N]
    mins1 = -np.maximum(m1[0::2], m1[1::2])    # [B, N] combine the two t-halves

    # exact argmin inside the winning 64-window, host fp32 arithmetic
    nearest_idx = np.empty((B, N), np.int64)
    offs = np.arange(WIN)
    for b in range(B):
        J = wstar[b][:, None] * WIN + offs[None, :]          # [N, WIN]
        pj = preds[b][J]                                     # [N, WIN, 3]
        Pwin = (
            xsq[b][:, None]
            + ysq[b][J]
            - 2.0 * np.einsum("nd,nwd->nw", gts[b], pj).astype(np.float32)
        )
        nearest_idx[b] = wstar[b] * WIN + Pwin.argmin(axis=1)

    loss_1 = mins1.astype(np.float64).mean()
    loss_2 = mins2.astype(np.float64).mean()
    chamfer = loss_1 + loss_2

    e0 = edges[:, 0]
    e1 = edges[:, 1]
    edge_vectors = preds[:, e0, :] - preds[:, e1, :]         # [B, E, 3]
    edge_loss = (edge_vectors * edge_vectors).sum(axis=2).astype(np.float64).mean()

    normals_nearest = np.take_along_axis(normals, nearest_idx[:, :, None], axis=1)
    normals_edge = normals_nearest[:, e0, :]                  # [B, E, 3]

    def l2n_dim1(v):
        n = np.sqrt((v * v).sum(axis=1, keepdims=True))
        return v / np.maximum(n, 1e-12)

    nn = l2n_dim1(normals_edge)
    nv = l2n_dim1(edge_vectors)
    cosines = np.abs((nn * nv).sum(axis=2))
    normal_cosine_loss = cosines.astype(np.float64).mean()

    return np.float32(
        30000.0 * chamfer + 240.0 * edge_loss + 200000.0 * normal_cosine_loss
    )


def kernel(preds, gts, normals, edges, _trace=False):
    from concourse.bass_utils import run_bass_kernel_spmd

    preds = np.asarray(preds, np.float32)
    gts = np.asarray(gts, np.float32)
    normals = np.asarray(normals, np.float32)
    edges = np.asarray(edges)

    nc = _build_nc()
    in_maps = _make_in_maps(preds, gts)
    br = run_bass_kernel_spmd(nc, in_maps, list(range(NCORES)), trace=_trace)
    _LAST_RESULTS["bass_results"] = br
    return _postprocess(preds, gts, normals, edges, br.results)


# revision 11
# speedup vs baseline: 1.9408x; 1.9408x over previous
"""Chamfer + edge + normal-cosine combined loss on 8 Trainium2 cores.

Device computes, per core (b = core//2, t-half h = core%2), the negated
distance matrix M[t,p] = 2<gts_t, preds_p> - |gts_t|^2 - |preds_p|^2 = -P[t,p]
via K=5 matmuls, then:
  - m64[t, w]   = max over p in 64-window w of M[t, :]   (row maxima -> mins2/argmin)
  - colmax[q,p] = max over t-tiles of M[t, p]            (-> mins1 after partition reduce)
Host finishes: mins2 = -max_w m64, argmin resolved exactly inside the winning
64-window, mins1 = -colmax reduced across the two t-halves, then the (tiny)
edge/normal-cosine losses in numpy.
"""

from contextlib import ExitStack

import ml_dtypes
import numpy as np

B = 4
N = 8192
NCORES = 8
TH = N // 2          # t rows per core
T_TILES = TH // 128  # 32
P_GROUPS = N // 2048  # 4
WIN = 64
NW = N // WIN        # 128 windows per row
K_SPLIT = 24         # bf16-split rows: 3 coords x 6 cross-terms + 3 xsq + 3 ysq
NEG_BIG = -3.0e38    # finite in bf16

_LAST_RESULTS = {}


def _split3(x):
    """Exact-ish 3-way bf16 decomposition of fp32: x ~ h + m + l (24 bits)."""
    h = x.astype(ml_dtypes.bfloat16)
    r1 = x - h.astype(np.float32)
    m = r1.astype(ml_dtypes.bfloat16)
    r2 = r1 - m.astype(np.float32)
    l = r2.astype(ml_dtypes.bfloat16)
    return h, m, l


def _build_split_rows(L, R):
    """L [5, X], R [5, Y] fp32 term rows -> bf16 [24, X], [24, Y].

    M = sum_k L[k] (outer) R[k]; each fp32 product is expanded into bf16
    cross-terms {hh, hm, mh, hl, lh, mm} (coords) or 3 terms (const rows)."""
    outL, outR = [], []
    for c in range(3):
        Lh, Lm, Ll = _split3(L[c])
        Rh, Rm, Rl = _split3(R[c])
        for a, b in ((Lh, Rh), (Lh, Rm), (Lm, Rh), (Lh, Rl), (Ll, Rh), (Lm, Rm)):
            outL.append(a)
            outR.append(b)
    Xh, Xm, Xl = _split3(L[3])
    negone = R[3].astype(ml_dtypes.bfloat16)
    for a in (Xh, Xm, Xl):
        outL.append(a)
        outR.append(negone)
    Yh, Ym, Yl = _split3(R[4])
    one = L[4].astype(ml_dtypes.bfloat16)
    for b in (Yh, Ym, Yl):
        outL.append(one)
        outR.append(b)
    return np.ascontiguousarray(np.stack(outL)), np.ascontiguousarray(np.stack(outR))


def _build_nc():
    import concourse.mybir as mybir
    import concourse.tile as tile
    from concourse import bacc, bass_isa

    f32 = mybir.dt.float32
    bf16 = mybir.dt.bfloat16
    nc = bacc.Bacc("TRN2", target_bir_lowering=False, debug=False)

    lhsT_d = nc.dram_tensor("lhsT", [K_SPLIT, TH], bf16, kind="ExternalInput")
    rhs_d = nc.dram_tensor("rhs", [K_SPLIT, N], bf16, kind="ExternalInput")
    m64_d = nc.dram_tensor("m64", [T_TILES, 128, NW], f32, kind="ExternalOutput")
    mins1_d = nc.dram_tensor("mins1neg", [1, N], bf16, kind="ExternalOutput")

    with tile.TileContext(nc) as tc, ExitStack() as ctx:
        const_pool = ctx.enter_context(tc.tile_pool(name="const", bufs=1))
        acc_pool = ctx.enter_context(tc.tile_pool(name="acc", bufs=1))
        stage_pool = ctx.enter_context(tc.tile_pool(name="stage", bufs=3))
        cpy_pool = ctx.enter_context(tc.tile_pool(name="cpy", bufs=3))
        psum_pool = ctx.enter_context(tc.tile_pool(name="psum", bufs=2, space="PSUM"))

        lhsT_s = const_pool.tile([K_SPLIT, TH], bf16)
        rhs_s = const_pool.tile([K_SPLIT, N], bf16)
        nc.sync.dma_start(lhsT_s[:], lhsT_d[:, :])
        nc.sync.dma_start(rhs_s[:], rhs_d[:, :])

        acc = acc_pool.tile([128, N], bf16)
        accr = acc_pool.tile([128, N], bf16)
        nc.vector.memset(acc[:], NEG_BIG)

        for i in range(T_TILES):
            w_i = lhsT_s[:, i * 128 : (i + 1) * 128]
            stage = stage_pool.tile([128, NW], f32)
            for g in range(P_GROUPS):
                ps = psum_pool.tile([128, 2048], f32)
                for c in range(4):
                    nc.tensor.matmul(
                        ps[:, c * 512 : (c + 1) * 512],
                        w_i,
                        rhs_s[:, g * 2048 + c * 512 : g * 2048 + (c + 1) * 512],
                        start=True,
                        stop=True,
                    )
                # per-64-window row maxima of this 2048-wide group (fp32, exact)
                nc.vector.tensor_reduce(
                    stage[:, g * 32 : (g + 1) * 32],
                    ps[:].rearrange("p (w e) -> p w e", e=WIN),
                    axis=mybir.AxisListType.X,
                    op=mybir.AluOpType.max,
                )
                # ACT casts the group to bf16 in SBUF; DVE runs the column
                # max in bf16 (2x mode) for mins1
                cpy = cpy_pool.tile([128, 2048], bf16)
                nc.scalar.copy(cpy[:], ps[:])
                nc.vector.tensor_max(
                    acc[:, g * 2048 : (g + 1) * 2048],
                    cpy[:],
                    acc[:, g * 2048 : (g + 1) * 2048],
                )
            nc.sync.dma_start(m64_d[i, :, :], stage[:])

        nc.gpsimd.partition_all_reduce(
            accr[:], acc[:], channels=128, reduce_op=bass_isa.ReduceOp.max
        )
        nc.sync.dma_start(mins1_d[:, :], accr[0:1, :])

    nc.compile()
    return nc


def _make_in_maps(preds, gts):
    xsq = np.sum(gts * gts, axis=-1)    # [B, N]
    ysq = np.sum(preds * preds, axis=-1)  # [B, N]
    in_maps = []
    for core in range(NCORES):
        b, h = divmod(core, 2)
        tsl = slice(h * TH, (h + 1) * TH)
        L = np.empty((5, TH), np.float32)
        L[0:3] = (2.0 * gts[b, tsl]).T
        L[3] = xsq[b, tsl]
        L[4] = 1.0
        R = np.empty((5, N), np.float32)
        R[0:3] = preds[b].T
        R[3] = -1.0
        R[4] = -ysq[b]
        sL, sR = _build_split_rows(L, R)
        in_maps.append({"lhsT": sL, "rhs": sR})
    return in_maps


def _postprocess(preds, gts, normals, edges, results):
    xsq = np.sum(gts * gts, axis=-1)
    ysq = np.sum(preds * preds, axis=-1)

    # m64 windows: core (b,h) tile i row q window w -> t = h*TH + i*128 + q
    m64 = np.stack([results[c]["m64"] for c in range(NCORES)])  # [8, 32, 128, NW]
    Mwin = m64.reshape(B, 2, T_TILES, 128, NW).reshape(B, N, NW)
    mins2 = -Mwin.max(axis=2)                  # [B, N]
    wstar = Mwin.argmax(axis=2)                # first occurrence

    m1 = np.stack(
        [np.asarray(results[c]["mins1neg"][0], np.float32) for c in range(NCORES)]
    )  # [8, N]
    mins1 = -np.maximum(m1[0::2], m1[1::2])    # [B, N] combine the two t-halves

    # exact argmin inside the winning 64-window, host fp32 arithmetic
    nearest_idx = np.empty((B, N), np.int64)
    offs = np.arange(WIN)
    for b in range(B):
        J = wstar[b][:, None] * WIN + offs[None, :]          # [N, WIN]
        pj = preds[b][J]                                     # [N, WIN, 3]
        Pwin = (
            xsq[b][:, None]
            + ysq[b][J]
            - 2.0 * np.einsum("nd,nwd->nw", gts[b], pj).astype(np.float32)
        )
        nearest_idx[b] = wstar[b] * WIN + Pwin.argmin(axis=1)

    loss_1 = mins1.astype(np.float64).mean()
    loss_2 = mins2.astype(np.float64).mean()
    chamfer = loss_1 + loss_2

    e0 = edges[:, 0]
    e1 = edges[:, 1]
    edge_vectors = preds[:, e0, :] - preds[:, e1, :]         # [B, E, 3]
    edge_loss = (edge_vectors * edge_vectors).sum(axis=2).astype(np.float64).mean()

    normals_nearest = np.take_along_axis(normals, nearest_idx[:, :, None], axis=1)
    normals_edge = normals_nearest[:, e0, :]                  # [B, E, 3]

    def l2n_dim1(v):
        n = np.sqrt((v * v).sum(axis=1, keepdims=True))
        return v / np.maximum(n, 1e-12)

    nn = l2n_dim1(normals_edge)
    nv = l2n_dim1(edge_vectors)
    cosines = np.abs((nn * nv).sum(axis=2))
    normal_cosine_loss = cosines.astype(np.float64).mean()

    return np.float32(
        30000.0 * chamfer + 240.0 * edge_loss + 200000.0 * normal_cosine_loss
    )


def kernel(preds, gts, normals, edges, _trace=False):
    from concourse.bass_utils import run_bass_kernel_spmd

    preds = np.asarray(preds, np.float32)
    gts = np.asarray(gts, np.float32)
    normals = np.asarray(normals, np.float32)
    edges = np.asarray(edges)

    nc = _build_nc()
    in_maps = _make_in_maps(preds, gts)
    br = run_bass_kernel_spmd(nc, in_maps, list(range(NCORES)), trace=_trace)
    _LAST_RESULTS["bass_results"] = br
    return _postprocess(preds, gts, normals, edges, br.results)


# revision 15
# speedup vs baseline: 2.7875x; 1.4362x over previous
"""Chamfer + edge + normal-cosine combined loss on 8 Trainium2 cores.

Device computes, per core (b = core//2, t-half h = core%2), the negated
distance matrix M[t,p] = 2<gts_t, preds_p> - |gts_t|^2 - |preds_p|^2 = -P[t,p]
via K=5 matmuls, then:
  - m64[t, w]   = max over p in 64-window w of M[t, :]   (row maxima -> mins2/argmin)
  - colmax[q,p] = max over t-tiles of M[t, p]            (-> mins1 after partition reduce)
Host finishes: mins2 = -max_w m64, argmin resolved exactly inside the winning
64-window, mins1 = -colmax reduced across the two t-halves, then the (tiny)
edge/normal-cosine losses in numpy.
"""

from contextlib import ExitStack

import ml_dtypes
import numpy as np

B = 4
N = 8192
NCORES = 8
TH = N // 2          # t rows per core
T_TILES = TH // 128  # 32
P_GROUPS = N // 2048  # 4
WIN = 64             # innermost window of the final reduce
NFOLD = 3            # row folds 8192->1024 before the reduce
NWIN = (N >> NFOLD) // WIN  # 16 window-maxima per row; window w covers
                            # columns {64w + 1024k + j : k<8, j<64}
K_SPLIT = 24         # bf16-split rows: 3 coords x 6 cross-terms + 3 xsq + 3 ysq
NEG_BIG = -3.0e38    # finite in bf16

_LAST_RESULTS = {}


def _split3(x):
    """Exact-ish 3-way bf16 decomposition of fp32: x ~ h + m + l (24 bits)."""
    h = x.astype(ml_dtypes.bfloat16)
    r1 = x - h.astype(np.float32)
    m = r1.astype(ml_dtypes.bfloat16)
    r2 = r1 - m.astype(np.float32)
    l = r2.astype(ml_dtypes.bfloat16)
    return h, m, l


def _build_split_rows(L, R):
    """L [5, X], R [5, Y] fp32 term rows -> bf16 [24, X], [24, Y].

    M = sum_k L[k] (outer) R[k]; each fp32 product is expanded into bf16
    cross-terms {hh, hm, mh, hl, lh, mm} (coords) or 3 terms (const rows)."""
    outL, outR = [], []
    for c in range(3):
        Lh, Lm, Ll = _split3(L[c])
        Rh, Rm, Rl = _split3(R[c])
        for a, b in ((Lh, Rh), (Lh, Rm), (Lm, Rh), (Lh, Rl), (Ll, Rh), (Lm, Rm)):
            outL.append(a)
            outR.append(b)
    Xh, Xm, Xl = _split3(L[3])
    negone = R[3].astype(ml_dtypes.bfloat16)
    for a in (Xh, Xm, Xl):
        outL.append(a)
        outR.append(negone)
    Yh, Ym, Yl = _split3(R[4])
    one = L[4].astype(ml_dtypes.bfloat16)
    for b in (Yh, Ym, Yl):
        outL.append(one)
        outR.append(b)
    return np.ascontiguousarray(np.stack(outL)), np.ascontiguousarray(np.stack(outR))


def _build_nc():
    import concourse.mybir as mybir
    import concourse.tile as tile
    from concourse import bacc

    f32 = mybir.dt.float32
    bf16 = mybir.dt.bfloat16
    nc = bacc.Bacc("TRN2", target_bir_lowering=False, debug=False)

    lhsT_d = nc.dram_tensor("lhsT", [K_SPLIT, TH], bf16, kind="ExternalInput")
    rhs_d = nc.dram_tensor("rhs", [K_SPLIT, N], bf16, kind="ExternalInput")
    m64_d = nc.dram_tensor("m64", [T_TILES, 128, NWIN], f32, kind="ExternalOutput")
    acc_d = nc.dram_tensor("accmax", [128, N], bf16, kind="ExternalOutput")

    with tile.TileContext(nc) as tc, ExitStack() as ctx:
        const_pool = ctx.enter_context(tc.tile_pool(name="const", bufs=1))
        acc_pool = ctx.enter_context(tc.tile_pool(name="acc", bufs=1))
        stage_pool = ctx.enter_context(tc.tile_pool(name="stage", bufs=3))
        cpy_pool = ctx.enter_context(tc.tile_pool(name="cpy", bufs=2))
        scr_pool = ctx.enter_context(tc.tile_pool(name="scr", bufs=2))
        psum_pool = ctx.enter_context(tc.tile_pool(name="psum", bufs=2, space="PSUM"))

        lhsT_s = const_pool.tile([K_SPLIT, TH], bf16)
        rhs_s = const_pool.tile([K_SPLIT, N], bf16)
        nc.sync.dma_start(lhsT_s[:], lhsT_d[:, :])
        nc.sync.dma_start(rhs_s[:], rhs_d[:, :])

        acc = acc_pool.tile([128, N], bf16)
        nc.gpsimd.memset(acc[:], NEG_BIG)

        for i in range(T_TILES):
            w_i = lhsT_s[:, i * 128 : (i + 1) * 128]
            stage = stage_pool.tile([128, NWIN], f32)
            rowcpy = cpy_pool.tile([128, N], bf16)
            for g in range(P_GROUPS):
                ps = psum_pool.tile([128, 2048], f32)
                for c in range(4):
                    nc.tensor.matmul(
                        ps[:, c * 512 : (c + 1) * 512],
                        w_i,
                        rhs_s[:, g * 2048 + c * 512 : g * 2048 + (c + 1) * 512],
                        start=True,
                        stop=True,
                    )
                # ACT is the sole PSUM consumer: casts the group into the
                # bf16 row copy (PE never stalls on DVE)
                nc.scalar.copy(rowcpy[:, g * 2048 : (g + 1) * 2048], ps[:])
            # DVE full-row work (all bf16, 2x mode for tt):
            # fold row 8192 -> 4096 -> 2048 -> 1024, then window-max 16x64
            s1 = scr_pool.tile([128, N // 2], bf16)
            nc.vector.tensor_max(s1[:], rowcpy[:, : N // 2], rowcpy[:, N // 2 :])
            s2 = scr_pool.tile([128, N // 4], bf16)
            nc.vector.tensor_max(s2[:], s1[:, : N // 4], s1[:, N // 4 :])
            s3 = scr_pool.tile([128, N // 8], bf16)
            nc.vector.tensor_max(s3[:], s2[:, : N // 8], s2[:, N // 8 :])
            nc.vector.tensor_reduce(
                stage[:],
                s3[:].rearrange("p (w e) -> p w e", e=WIN),
                axis=mybir.AxisListType.X,
                op=mybir.AluOpType.max,
            )
            # column-direction running max for mins1 (one wide bf16 2x op)
            nc.vector.tensor_max(acc[:], rowcpy[:], acc[:])
            nc.sync.dma_start(m64_d[i, :, :], stage[:])

        nc.sync.dma_start(acc_d[:, :], acc[:])

    nc.compile()
    return nc


def _make_in_maps(preds, gts):
    xsq = np.sum(gts * gts, axis=-1)    # [B, N]
    ysq = np.sum(preds * preds, axis=-1)  # [B, N]
    in_maps = []
    for core in range(NCORES):
        b, h = divmod(core, 2)
        tsl = slice(h * TH, (h + 1) * TH)
        L = np.empty((5, TH), np.float32)
        L[0:3] = (2.0 * gts[b, tsl]).T
        L[3] = xsq[b, tsl]
        L[4] = 1.0
        R = np.empty((5, N), np.float32)
        R[0:3] = preds[b].T
        R[3] = -1.0
        R[4] = -ysq[b]
        sL, sR = _build_split_rows(L, R)
        in_maps.append({"lhsT": sL, "rhs": sR})
    return in_maps


def _postprocess(preds, gts, normals, edges, results):
    xsq = np.sum(gts * gts, axis=-1)
    ysq = np.sum(preds * preds, axis=-1)

    # window maxima: core (b,h) tile i row q window w -> t = h*TH + i*128 + q.
    # Values are bf16-quantized (device folds in bf16), so a row's true best
    # window is always among the windows whose quantized max equals the row
    # max; resolve those candidates exactly in fp32 on the host.
    m64 = np.stack([results[c]["m64"] for c in range(NCORES)])  # [8, 32, 128, NWIN]
    Mwin = m64.reshape(B, 2, T_TILES, 128, NWIN).reshape(B, N, NWIN)
    mx = Mwin.max(axis=2, keepdims=True)
    iscand = Mwin == mx                        # [B, N, NWIN]

    m1 = np.stack(
        [np.asarray(results[c]["accmax"], np.float32) for c in range(NCORES)]
    )  # [8, 128, N]
    m1 = m1.max(axis=1)                        # [8, N] per-core column max
    mins1 = -np.maximum(m1[0::2], m1[1::2])    # [B, N] combine the two t-halves

    # candidate windows -> exact fp32 min + first-occurrence argmin
    spans = np.arange(N >> NFOLD, dtype=np.int64).reshape(1, -1)  # 1024 cols
    nearest_idx = np.empty((B, N), np.int64)
    mins2 = np.empty((B, N), np.float32)
    best_val = np.full((B, N), np.float32(np.inf))
    best_idx = np.full((B, N), np.int64(N))
    kk = np.arange(1 << NFOLD, dtype=np.int64) * (N >> NFOLD)     # 8 folds
    offs = np.arange(WIN, dtype=np.int64)
    for w in range(NWIN):
        Jw = (w * WIN + offs[None, :] + kk[:, None]).ravel()      # [512] ascending
        Jw.sort()
        for b in range(B):
            rows = np.nonzero(iscand[b, :, w])[0]
            if rows.size == 0:
                continue
            pj = preds[b][Jw]                                     # [512, 3]
            Pw = (
                xsq[b][rows, None]
                + ysq[b][Jw][None, :]
                - 2.0 * (gts[b][rows] @ pj.T).astype(np.float32)
            )
            mn = Pw.min(axis=1)
            am = Jw[Pw.argmin(axis=1)]
            bv = best_val[b, rows]
            bi = best_idx[b, rows]
            upd = (mn < bv) | ((mn == bv) & (am < bi))
            best_val[b, rows] = np.where(upd, mn, bv)
            best_idx[b, rows] = np.where(upd, am, bi)
    mins2 = best_val
    nearest_idx = best_idx

    loss_1 = mins1.astype(np.float64).mean()
    loss_2 = mins2.astype(np.float64).mean()
    chamfer = loss_1 + loss_2

    e0 = edges[:, 0]
    e1 = edges[:, 1]
    edge_vectors = preds[:, e0, :] - preds[:, e1, :]         # [B, E, 3]
    edge_loss = (edge_vectors * edge_vectors).sum(axis=2).astype(np.float64).mean()

    normals_nearest = np.take_along_axis(normals, nearest_idx[:, :, None], axis=1)
    normals_edge = normals_nearest[:, e0, :]                  # [B, E, 3]

    def l2n_dim1(v):
        n = np.sqrt((v * v).sum(axis=1, keepdims=True))
        return v / np.maximum(n, 1e-12)

    nn = l2n_dim1(normals_edge)
    nv = l2n_dim1(edge_vectors)
    cosines = np.abs((nn * nv).sum(axis=2))
    normal_cosine_loss = cosines.astype(np.float64).mean()

    return np.float32(
        30000.0 * chamfer + 240.0 * edge_loss + 200000.0 * normal_cosine_loss
    )


def kernel(preds, gts, normals, edges, _trace=False):
    from concourse.bass_utils import run_bass_kernel_spmd

    preds = np.asarray(preds, np.float32)
    gts = np.asarray(gts, np.float32)
    normals = np.asarray(normals, np.float32)
    edges = np.asarray(edges)

    nc = _build_nc()
    in_maps = _make_in_maps(preds, gts)
    br = run_bass_kernel_spmd(nc, in_maps, list(range(NCORES)), trace=_trace)
    _LAST_RESULTS["bass_results"] = br
    return _postprocess(preds, gts, normals, edges, br.results)


# revision 18
# speedup vs baseline: 2.8285x; 1.0147x over previous
"""Chamfer + edge + normal-cosine combined loss on 8 Trainium2 cores.

Device computes, per core (b = core//2, t-half h = core%2), the negated
distance matrix M[t,p] = 2<gts_t, preds_p> - |gts_t|^2 - |preds_p|^2 = -P[t,p]
via K=5 matmuls, then:
  - m64[t, w]   = max over p in 64-window w of M[t, :]   (row maxima -> mins2/argmin)
  - colmax[q,p] = max over t-tiles of M[t, p]            (-> mins1 after partition reduce)
Host finishes: mins2 = -max_w m64, argmin resolved exactly inside the winning
64-window, mins1 = -colmax reduced across the two t-halves, then the (tiny)
edge/normal-cosine losses in numpy.
"""

from contextlib import ExitStack

import ml_dtypes
import numpy as np

B = 4
N = 8192
NCORES = 8
TH = N // 2          # t rows per core
T_TILES = TH // 128  # 32
P_GROUPS = N // 2048  # 4
WIN = 64             # innermost window of the final reduce
NFOLD = 4            # row folds 8192->512 before the reduce
NWIN = (N >> NFOLD) // WIN  # 16 window-maxima per row; window w covers
                            # columns {64w + 1024k + j : k<8, j<64}
K_SPLIT = 24         # bf16-split rows: 3 coords x 6 cross-terms + 3 xsq + 3 ysq
NEG_BIG = -3.0e38    # finite in bf16

_LAST_RESULTS = {}


def _split3(x):
    """Exact-ish 3-way bf16 decomposition of fp32: x ~ h + m + l (24 bits)."""
    h = x.astype(ml_dtypes.bfloat16)
    r1 = x - h.astype(np.float32)
    m = r1.astype(ml_dtypes.bfloat16)
    r2 = r1 - m.astype(np.float32)
    l = r2.astype(ml_dtypes.bfloat16)
    return h, m, l


def _build_split_rows(L, R):
    """L [5, X], R [5, Y] fp32 term rows -> bf16 [24, X], [24, Y].

    M = sum_k L[k] (outer) R[k]; each fp32 product is expanded into bf16
    cross-terms {hh, hm, mh, hl, lh, mm} (coords) or 3 terms (const rows)."""
    outL, outR = [], []
    for c in range(3):
        Lh, Lm, Ll = _split3(L[c])
        Rh, Rm, Rl = _split3(R[c])
        for a, b in ((Lh, Rh), (Lh, Rm), (Lm, Rh), (Lh, Rl), (Ll, Rh), (Lm, Rm)):
            outL.append(a)
            outR.append(b)
    Xh, Xm, Xl = _split3(L[3])
    negone = R[3].astype(ml_dtypes.bfloat16)
    for a in (Xh, Xm, Xl):
        outL.append(a)
        outR.append(negone)
    Yh, Ym, Yl = _split3(R[4])
    one = L[4].astype(ml_dtypes.bfloat16)
    for b in (Yh, Ym, Yl):
        outL.append(one)
        outR.append(b)
    return np.ascontiguousarray(np.stack(outL)), np.ascontiguousarray(np.stack(outR))


def _build_nc():
    import concourse.mybir as mybir
    import concourse.tile as tile
    from concourse import bacc

    f32 = mybir.dt.float32
    bf16 = mybir.dt.bfloat16
    nc = bacc.Bacc("TRN2", target_bir_lowering=False, debug=False)

    lhsT_d = nc.dram_tensor("lhsT", [K_SPLIT, TH], bf16, kind="ExternalInput")
    rhs_d = nc.dram_tensor("rhs", [K_SPLIT, N], bf16, kind="ExternalInput")
    m64_d = nc.dram_tensor("m64", [T_TILES, 128, NWIN], f32, kind="ExternalOutput")
    acc_d = nc.dram_tensor("accmax", [128, N], bf16, kind="ExternalOutput")

    with tile.TileContext(nc) as tc, ExitStack() as ctx:
        const_pool = ctx.enter_context(tc.tile_pool(name="const", bufs=1))
        acc_pool = ctx.enter_context(tc.tile_pool(name="acc", bufs=1))
        stage_pool = ctx.enter_context(tc.tile_pool(name="stage", bufs=3))
        cpy_pool = ctx.enter_context(tc.tile_pool(name="cpy", bufs=3))
        scr_pool = ctx.enter_context(tc.tile_pool(name="scr", bufs=2))
        psum_pool = ctx.enter_context(tc.tile_pool(name="psum", bufs=4, space="PSUM"))

        lhsT_s = const_pool.tile([K_SPLIT, TH], bf16)
        rhs_s = const_pool.tile([K_SPLIT, N], bf16)
        nc.sync.dma_start(lhsT_s[:], lhsT_d[:, :])
        nc.sync.dma_start(rhs_s[:], rhs_d[:, :])

        acc = acc_pool.tile([128, N], bf16)
        nc.gpsimd.memset(acc[:], NEG_BIG)

        for i in range(T_TILES):
            w_i = lhsT_s[:, i * 128 : (i + 1) * 128]
            stage = stage_pool.tile([128, NWIN], f32)
            rowcpy = cpy_pool.tile([128, N], bf16)
            for g in range(N // 1024):
                ps = psum_pool.tile([128, 1024], f32)
                for c in range(2):
                    nc.tensor.matmul(
                        ps[:, c * 512 : (c + 1) * 512],
                        w_i,
                        rhs_s[:, g * 1024 + c * 512 : g * 1024 + (c + 1) * 512],
                        start=True,
                        stop=True,
                    )
                # ACT is the sole PSUM consumer: casts the group into the
                # bf16 row copy (PE never stalls on DVE)
                nc.scalar.copy(rowcpy[:, g * 1024 : (g + 1) * 1024], ps[:])
            # DVE full-row work (all bf16, 2x mode for tt):
            # fold row 8192 -> 4096 -> 2048 -> 1024 -> 512, then window-max 8x64
            s1 = scr_pool.tile([128, N // 2], bf16)
            nc.vector.tensor_max(s1[:], rowcpy[:, : N // 2], rowcpy[:, N // 2 :])
            s2 = scr_pool.tile([128, N // 4], bf16)
            nc.vector.tensor_max(s2[:], s1[:, : N // 4], s1[:, N // 4 :])
            s3 = scr_pool.tile([128, N // 8], bf16)
            nc.vector.tensor_max(s3[:], s2[:, : N // 8], s2[:, N // 8 :])
            s4 = scr_pool.tile([128, N // 16], bf16)
            nc.vector.tensor_max(s4[:], s3[:, : N // 16], s3[:, N // 16 :])
            nc.vector.tensor_reduce(
                stage[:],
                s4[:].rearrange("p (w e) -> p w e", e=WIN),
                axis=mybir.AxisListType.X,
                op=mybir.AluOpType.max,
            )
            # column-direction running max for mins1 (one wide bf16 2x op)
            nc.vector.tensor_max(acc[:], rowcpy[:], acc[:])
            nc.sync.dma_start(m64_d[i, :, :], stage[:])

        nc.sync.dma_start(acc_d[:, :], acc[:])

    nc.compile()
    return nc


def _make_in_maps(preds, gts):
    xsq = np.sum(gts * gts, axis=-1)    # [B, N]
    ysq = np.sum(preds * preds, axis=-1)  # [B, N]
    in_maps = []
    for core in range(NCORES):
        b, h = divmod(core, 2)
        tsl = slice(h * TH, (h + 1) * TH)
        L = np.empty((5, TH), np.float32)
        L[0:3] = (2.0 * gts[b, tsl]).T
        L[3] = xsq[b, tsl]
        L[4] = 1.0
        R = np.empty((5, N), np.float32)
        R[0:3] = preds[b].T
        R[3] = -1.0
        R[4] = -ysq[b]
        sL, sR = _build_split_rows(L, R)
        in_maps.append({"lhsT": sL, "rhs": sR})
    return in_maps


def _postprocess(preds, gts, normals, edges, results):
    xsq = np.sum(gts * gts, axis=-1)
    ysq = np.sum(preds * preds, axis=-1)

    # window maxima: core (b,h) tile i row q window w -> t = h*TH + i*128 + q.
    # Values are bf16-quantized (device folds in bf16), so a row's true best
    # window is always among the windows whose quantized max equals the row
    # max; resolve those candidates exactly in fp32 on the host.
    m64 = np.stack([results[c]["m64"] for c in range(NCORES)])  # [8, 32, 128, NWIN]
    Mwin = m64.reshape(B, 2, T_TILES, 128, NWIN).reshape(B, N, NWIN)
    mx = Mwin.max(axis=2, keepdims=True)
    iscand = Mwin == mx                        # [B, N, NWIN]

    m1 = np.stack(
        [np.asarray(results[c]["accmax"], np.float32) for c in range(NCORES)]
    )  # [8, 128, N]
    m1 = m1.max(axis=1)                        # [8, N] per-core column max
    mins1 = -np.maximum(m1[0::2], m1[1::2])    # [B, N] combine the two t-halves

    # candidate windows -> exact fp32 min + first-occurrence argmin
    spans = np.arange(N >> NFOLD, dtype=np.int64).reshape(1, -1)  # 1024 cols
    nearest_idx = np.empty((B, N), np.int64)
    mins2 = np.empty((B, N), np.float32)
    best_val = np.full((B, N), np.float32(np.inf))
    best_idx = np.full((B, N), np.int64(N))
    kk = np.arange(1 << NFOLD, dtype=np.int64) * (N >> NFOLD)     # 8 folds
    offs = np.arange(WIN, dtype=np.int64)
    for w in range(NWIN):
        Jw = (w * WIN + offs[None, :] + kk[:, None]).ravel()      # [512] ascending
        Jw.sort()
        for b in range(B):
            rows = np.nonzero(iscand[b, :, w])[0]
            if rows.size == 0:
                continue
            pj = preds[b][Jw]                                     # [512, 3]
            Pw = (
                xsq[b][rows, None]
                + ysq[b][Jw][None, :]
                - 2.0 * (gts[b][rows] @ pj.T).astype(np.float32)
            )
            mn = Pw.min(axis=1)
            am = Jw[Pw.argmin(axis=1)]
            bv = best_val[b, rows]
            bi = best_idx[b, rows]
            upd = (mn < bv) | ((mn == bv) & (am < bi))
            best_val[b, rows] = np.where(upd, mn, bv)
            best_idx[b, rows] = np.where(upd, am, bi)
    mins2 = best_val
    nearest_idx = best_idx

    loss_1 = mins1.astype(np.float64).mean()
    loss_2 = mins2.astype(np.float64).mean()
    chamfer = loss_1 + loss_2

    e0 = edges[:, 0]
    e1 = edges[:, 1]
    edge_vectors = preds[:, e0, :] - preds[:, e1, :]         # [B, E, 3]
    edge_loss = (edge_vectors * edge_vectors).sum(axis=2).astype(np.float64).mean()

    normals_nearest = np.take_along_axis(normals, nearest_idx[:, :, None], axis=1)
    normals_edge = normals_nearest[:, e0, :]                  # [B, E, 3]

    def l2n_dim1(v):
        n = np.sqrt((v * v).sum(axis=1, keepdims=True))
        return v / np.maximum(n, 1e-12)

    nn = l2n_dim1(normals_edge)
    nv = l2n_dim1(edge_vectors)
    cosines = np.abs((nn * nv).sum(axis=2))
    normal_cosine_loss = cosines.astype(np.float64).mean()

    return np.float32(
        30000.0 * chamfer + 240.0 * edge_loss + 200000.0 * normal_cosine_loss
    )


def kernel(preds, gts, normals, edges, _trace=False):
    from concourse.bass_utils import run_bass_kernel_spmd

    preds = np.asarray(preds, np.float32)
    gts = np.asarray(gts, np.float32)
    normals = np.asarray(normals, np.float32)
    edges = np.asarray(edges)

    nc = _build_nc()
    in_maps = _make_in_maps(preds, gts)
    br = run_bass_kernel_spmd(nc, in_maps, list(range(NCORES)), trace=_trace)
    _LAST_RESULTS["bass_results"] = br
    return _postprocess(preds, gts, normals, edges, br.results)
